# revision 16
# baseline (speedup 1.0000x reference)
"""Distributed Bass kernel for sliding-window GQA attention on 8 TRN2 NeuronCores.

Problem: B=2, S=2048, DIM=2048, H=16, KVH=4, HD=128, WINDOW=1024 (causal
sliding window), nonstandard RoPE producing 1.5*HD score features.

Sharding (tensor-parallel on the kv-head axis, data-parallel on batch —
no collectives): core c owns (batch, kv-group) = (c//4, c%4): its 4 q-heads
and 1 kv head over the full 2048-row sequence. wq/wk/wv are column-sharded
by kv group, wo row-sharded. Each core emits a PARTIAL output projection
(its 4 heads x its wo rows); the host sums the 4 partials per batch while
unsharding — replacing the all-reduce.

Per core: Q/K/V projections + rope scaling, block-sparse sliding-window
attention in global coordinates (k-blocks max(0,qc-8)..qc per 128-row
q-block qc), unnormalized exp softmax (bounded scores, no max pass), a
transpose-by-matmul against diag(1/rowsum) that normalizes for free, PV,
and the partial O-projection.
"""
import numpy as np
import ml_dtypes

import concourse.tile as tile
from concourse import bacc, mybir
from concourse.bass_utils import run_bass_kernel_spmd
from contextlib import ExitStack

F32 = mybir.dt.float32
BF16 = mybir.dt.bfloat16
EXP = mybir.ActivationFunctionType.Exp

B, S, DIM = 2, 2048, 2048
H, KVH, HD = 16, 4, 128
HPC = H // KVH  # heads per core (4)
WINDOW = 1024
SCALE = HD ** -0.5
NDC = DIM // 128  # 16 dim chunks
NQC = S // 128    # 16 q blocks

_cache = {}


def _kblocks(qc):
    return list(range(max(0, qc - 8), qc + 1))


def _build():
    nc = bacc.Bacc("TRN2", target_bir_lowering=False, debug=False, num_devices=8)

    xt_d = nc.dram_tensor("xt", [128, 4 * NDC * 512], BF16, kind="ExternalInput")
    wq_d = nc.dram_tensor("wq", [128, 2 * NDC * 256], BF16, kind="ExternalInput")
    wkv_d = nc.dram_tensor("wkv", [128, NDC * 256], BF16, kind="ExternalInput")
    wo_d = nc.dram_tensor("wo", [128, 2 * 2 * 2048], BF16, kind="ExternalInput")
    fm_d = nc.dram_tensor("fm", [64, S], F32, kind="ExternalInput")
    fp_d = nc.dram_tensor("fp", [64, S], F32, kind="ExternalInput")
    t0_d = nc.dram_tensor("t0", [128, 128], F32, kind="ExternalInput")
    t8_d = nc.dram_tensor("t8", [128, 128], F32, kind="ExternalInput")
    id_d = nc.dram_tensor("ident", [128, 128], BF16, kind="ExternalInput")
    out_d = nc.dram_tensor("out", [S, DIM], F32, kind="ExternalOutput")

    with tile.TileContext(nc) as tc, ExitStack() as ctx:
        xp = ctx.enter_context(tc.tile_pool(name="xp", bufs=2))
        wp = ctx.enter_context(tc.tile_pool(name="wp", bufs=5))
        cp = ctx.enter_context(tc.tile_pool(name="cp", bufs=1))
        qp = ctx.enter_context(tc.tile_pool(name="qp", bufs=1))
        kp = ctx.enter_context(tc.tile_pool(name="kp", bufs=1))
        vp = ctx.enter_context(tc.tile_pool(name="vp", bufs=1))
        pp = ctx.enter_context(tc.tile_pool(name="pp", bufs=2))
        ptp = ctx.enter_context(tc.tile_pool(name="ptp", bufs=2))
        dgp = ctx.enter_context(tc.tile_pool(name="dgp", bufs=2))
        smp = ctx.enter_context(tc.tile_pool(name="smp", bufs=8))
        ap_ = ctx.enter_context(tc.tile_pool(name="ap", bufs=1))
        op_ = ctx.enter_context(tc.tile_pool(name="op", bufs=2))
        ps = ctx.enter_context(tc.tile_pool(name="ps", bufs=4, space="PSUM"))
        sps = ctx.enter_context(tc.tile_pool(name="sps", bufs=4, space="PSUM"))

        # ---- weights for phase 1 first (prologue-critical DMA order) ----
        wkv_t = wp.tile([128, NDC, 256], BF16, tag="w")  # cols: [wk 128 | wv 128]
        nc.sync.dma_start(wkv_t[:], wkv_d[:, :])
        wq_t = None  # allocated after the first x chunk's DMAs

        q1 = qp.tile([128, HPC, S], BF16, tag="q1")
        q2 = qp.tile([64, HPC, S], BF16, tag="q2")
        k1 = kp.tile([128, S], BF16, tag="k1")
        k2 = kp.tile([64, S], BF16, tag="k2")
        v_sb = vp.tile([128, NQC, 128], BF16, tag="v")

        # ---- fused projections + attention + O-proj per column-quarter ----
        fm = fp = t0 = t8 = ident = wo_t = None
        attn = ap_.tile([128, HPC, S], BF16, tag="attn")
        for cq in range(4):
            x_q = xp.tile([128, NDC, 512], BF16, tag="x")
            for dg in range(4):
                nc.sync.dma_start(
                    x_q[:, dg * 4 : (dg + 1) * 4, :],
                    xt_d[
                        :,
                        cq * NDC * 512 + dg * 4 * 512 : cq * NDC * 512
                        + (dg + 1) * 4 * 512,
                    ],
                )
            if cq == 0:
                # wq + constants ride after the first x chunk (not prologue-critical)
                wq_t = [
                    wp.tile([128, NDC, 256], BF16, tag="w", name=f"wq{i}")
                    for i in range(2)
                ]
                for i in range(2):
                    nc.sync.dma_start(
                        wq_t[i][:], wq_d[:, i * NDC * 256 : (i + 1) * NDC * 256]
                    )
                fm = cp.tile([64, S], F32, tag="fm")
                nc.sync.dma_start(fm[:], fm_d[:, :])
                fp = cp.tile([64, S], F32, tag="fp")
                nc.sync.dma_start(fp[:], fp_d[:, :])
                t0 = cp.tile([128, 128], F32, tag="t0")
                nc.sync.dma_start(t0[:], t0_d[:, :])
                t8 = cp.tile([128, 128], F32, tag="t8")
                nc.sync.dma_start(t8[:], t8_d[:, :])
                ident = cp.tile([128, 128], BF16, tag="ident")
                nc.sync.dma_start(ident[:], id_d[:, :])
                wo_t = [
                    wp.tile([128, 2, 2048], BF16, tag="w", name=f"wo{i}")
                    for i in range(2)
                ]
                for i in range(2):
                    nc.sync.dma_start(wo_t[i][:], wo_d[:, i * 4096 : (i + 1) * 4096])
            cs = slice(cq * 512, (cq + 1) * 512)
            fmc, fpc = fm[:, cs], fp[:, cs]

            kps = ps.tile([128, 512], F32, tag="ps")
            for dc in range(NDC):
                nc.tensor.matmul(
                    kps[:],
                    wkv_t[:, dc, 0:128],
                    x_q[:, dc, :],
                    start=(dc == 0),
                    stop=(dc == NDC - 1),
                )
            nc.vector.tensor_mul(k1[0:64, cs], kps[0:64, :], fmc)
            nc.vector.tensor_mul(k1[64:128, cs], kps[0:64, :], fpc)
            nc.scalar.copy(k2[:, cs], kps[64:128, :])

            vps = ps.tile([128, 512], F32, tag="ps")
            for kb4 in range(4):
                kb = cq * 4 + kb4
                for dc in range(NDC):
                    nc.tensor.matmul(
                        vps[:, kb4 * 128 : (kb4 + 1) * 128],
                        x_q[:, dc, kb4 * 128 : (kb4 + 1) * 128],
                        wkv_t[:, dc, 128:256],
                        start=(dc == 0),
                        stop=(dc == NDC - 1),
                    )
            for kb4 in range(4):
                nc.any.tensor_copy(
                    v_sb[:, cq * 4 + kb4, :], vps[:, kb4 * 128 : (kb4 + 1) * 128]
                )

            for h in range(HPC):
                qps = ps.tile([128, 512], F32, tag="ps")
                for dc in range(NDC):
                    nc.tensor.matmul(
                        qps[:],
                        wq_t[h // 2][:, dc, (h % 2) * 128 : (h % 2 + 1) * 128],
                        x_q[:, dc, :],
                        start=(dc == 0),
                        stop=(dc == NDC - 1),
                    )
                nc.vector.tensor_mul(q1[0:64, h, cs], qps[0:64, :], fmc)
                nc.vector.tensor_mul(q1[64:128, h, cs], qps[0:64, :], fpc)
                nc.scalar.copy(q2[:, h, cs], qps[64:128, :])

            # ---- attention for q-group qg=cq + interleaved partial O-proj ----
            qg = cq
            for h in range(HPC):
                aps = ps.tile([128, 512], F32, tag="ps")
                for qc4 in range(4):
                    qc = qg * 4 + qc4
                    qb = qc * 128
                    kbs = _kblocks(qc)
                    nkb = len(kbs)
                    chunks = [kbs[i : i + 3] for i in range(0, nkb, 3)]
                    schunks = []
                    for chunk in chunks:
                        w = len(chunk) * 128
                        sp = sps.tile([128, 384], F32, tag="s")
                        lo = chunk[0] * 128
                        nc.tensor.matmul(
                            sp[:, 0:w],
                            q1[:, h, qb : qb + 128],
                            k1[:, lo : lo + w],
                            start=True,
                            stop=False,
                        )
                        nc.tensor.matmul(
                            sp[:, 0:w],
                            q2[:, h, qb : qb + 128],
                            k2[:, lo : lo + w],
                            start=False,
                            stop=True,
                        )
                        schunks.append(sp)
                    # masks: window-tail triangle on k-block qc-8, causal on qc
                    if kbs[0] == qc - 8:
                        nc.vector.tensor_add(
                            schunks[0][:, 0:128], schunks[0][:, 0:128], t0[:]
                        )
                    dpos = (nkb - 1) % 3
                    nc.vector.tensor_add(
                        schunks[-1][:, dpos * 128 : (dpos + 1) * 128],
                        schunks[-1][:, dpos * 128 : (dpos + 1) * 128],
                        t8[:],
                    )
                    # exp + row sums
                    p_sb = pp.tile([128, 1152], BF16, tag="p")
                    acc = smp.tile([128, 3], F32, tag="acc")
                    for ci, chunk in enumerate(chunks):
                        w = len(chunk) * 128
                        nc.scalar.activation(
                            p_sb[:, ci * 384 : ci * 384 + w],
                            schunks[ci][:, 0:w],
                            EXP,
                            accum_out=acc[:, ci : ci + 1],
                        )
                    sm = smp.tile([128, 1], F32, tag="sm")
                    if len(chunks) == 1:
                        nc.vector.tensor_copy(sm[:], acc[:, 0:1])
                    else:
                        nc.vector.tensor_add(sm[:], acc[:, 0:1], acc[:, 1:2])
                        if len(chunks) == 3:
                            nc.vector.tensor_add(sm[:], sm[:], acc[:, 2:3])
                    rc = smp.tile([128, 1], F32, tag="rc")
                    nc.vector.reciprocal(rc[:], sm[:])
                    dg = dgp.tile([128, 128], BF16, tag="dg")
                    nc.vector.tensor_scalar_mul(dg[:], ident[:], rc[:, 0:1])
                    # normalized transpose: PT[k,q] = P^T @ diag(1/sum)
                    pt_sb = ptp.tile([128, 1152], BF16, tag="pt")
                    for ci, chunk in enumerate(chunks):
                        w = len(chunk) * 128
                        ptps = sps.tile([128, 384], F32, tag="s")
                        for t in range(len(chunk)):
                            nc.tensor.matmul(
                                ptps[:, t * 128 : (t + 1) * 128],
                                p_sb[:, ci * 384 + t * 128 : ci * 384 + (t + 1) * 128],
                                dg[:],
                                start=True,
                                stop=True,
                            )
                        nc.any.tensor_copy(
                            pt_sb[:, ci * 384 : ci * 384 + w], ptps[:, 0:w]
                        )
                    # PV
                    for mi, kb in enumerate(kbs):
                        ci, t = mi // 3, mi % 3
                        nc.tensor.matmul(
                            aps[:, qc4 * 128 : (qc4 + 1) * 128],
                            v_sb[:, kb, :],
                            pt_sb[:, ci * 384 + t * 128 : ci * 384 + (t + 1) * 128],
                            start=(mi == 0),
                            stop=(mi == nkb - 1),
                        )
                nc.any.tensor_copy(attn[:, h, qg * 512 : (qg + 1) * 512], aps[:])

            # partial O-projection for this q-group (overlaps next group's attn)
            for qc in range(qg * 4, (qg + 1) * 4):
                o_sb = op_.tile([128, 2048], F32, tag="o")
                for dn in range(4):
                    ops = ps.tile([128, 512], F32, tag="ps")
                    for f in range(HPC):
                        nc.tensor.matmul(
                            ops[:],
                            attn[:, f, qc * 128 : (qc + 1) * 128],
                            wo_t[f // 2][:, f % 2, dn * 512 : (dn + 1) * 512],
                            start=(f == 0),
                            stop=(f == HPC - 1),
                        )
                    nc.any.tensor_copy(o_sb[:, dn * 512 : (dn + 1) * 512], ops[:])
                nc.sync.dma_start(out_d[qc * 128 : (qc + 1) * 128, :], o_sb[:])

    nc.compile()
    return nc


def _prep_core(inputs, c):
    x = inputs["x"]
    cos, sin = np.asarray(inputs["cos"]), np.asarray(inputs["sin"])
    mask = np.asarray(inputs["mask"])
    wq = np.asarray(inputs["wq"], dtype=np.float32)
    wk = np.asarray(inputs["wk"], dtype=np.float32)
    wv = np.asarray(inputs["wv"], dtype=np.float32)
    wo = np.asarray(inputs["wo"], dtype=np.float32)
    bf = ml_dtypes.bfloat16
    b, g = c // 4, c % 4

    # x[b] transposed -> [128p, cq, dc, 512]
    xt = np.asarray(x[b], dtype=np.float32).T  # [dim, S]
    xt = xt.reshape(NDC, 128, 4, 512).transpose(1, 2, 0, 3)
    xt = np.ascontiguousarray(xt).reshape(128, 4 * NDC * 512).astype(bf)

    # wq slice for heads 4g..4g+3 (SCALE folded), [p, hpair, dc, 256]
    wqs = (wq[:, g * 512 : (g + 1) * 512] * SCALE).reshape(NDC, 128, 2, 256)
    wqs = np.ascontiguousarray(wqs.transpose(1, 2, 0, 3)).reshape(128, 2 * NDC * 256)
    # wk|wv slice for kv head g: [p, dc, 256] with cols [wk 128 | wv 128]
    wkv = np.concatenate(
        [wk[:, g * 128 : (g + 1) * 128], wv[:, g * 128 : (g + 1) * 128]], axis=1
    )
    wkv = np.ascontiguousarray(wkv.reshape(NDC, 128, 256).transpose(1, 0, 2)).reshape(
        128, NDC * 256
    )
    # wo rows for this core's heads: [p, f2(2 within pair), ...] tiles [128,2,2048]
    wos = wo[g * 512 : (g + 1) * 512].reshape(2, 2, 128, 2048).transpose(2, 0, 1, 3)
    wos = np.ascontiguousarray(wos).reshape(128, 2 * 2 * 2048)

    fm = np.ascontiguousarray((cos - sin).T, dtype=np.float32)
    fp_ = np.ascontiguousarray((cos + sin).T, dtype=np.float32)
    t0 = np.ascontiguousarray(mask[WINDOW : WINDOW + 128, 0:128], dtype=np.float32)
    t8 = np.ascontiguousarray(mask[0:128, 0:128], dtype=np.float32)

    return {
        "xt": xt, "wq": wqs.astype(bf), "wkv": wkv.astype(bf), "wo": wos.astype(bf),
        "fm": fm, "fp": fp_, "t0": t0, "t8": t8,
        "ident": np.eye(128, dtype=np.float32).astype(bf),
    }


def kernel(**inputs) -> np.ndarray:
    if "nc" not in _cache:
        _cache["nc"] = _build()
    nc = _cache["nc"]
    in_maps = [_prep_core(inputs, c) for c in range(8)]
    res = run_bass_kernel_spmd(nc, in_maps, core_ids=list(range(8)))
    out = np.zeros((B, S, DIM), dtype=np.float32)
    for c in range(8):
        out[c // 4] += res.results[c]["out"]
    return out


# revision 17
# speedup vs baseline: 1.4714x; 1.4714x over previous
"""Distributed Bass kernel for sliding-window GQA attention on 8 TRN2 NeuronCores.

Problem: B=2, S=2048, DIM=2048, H=16, KVH=4, HD=128, WINDOW=1024 (causal
sliding window), nonstandard RoPE producing 1.5*HD score features.

Sharding (tensor-parallel on the kv-head axis, data-parallel on batch —
no collectives): core c owns (batch, kv-group) = (c//4, c%4): its 4 q-heads
and 1 kv head over the full 2048-row sequence. wq/wk/wv are column-sharded
by kv group, wo row-sharded. Each core emits a PARTIAL output projection
(its 4 heads x its wo rows); the host sums the 4 partials per batch while
unsharding — replacing the all-reduce.

Per core: Q/K/V projections + rope scaling, block-sparse sliding-window
attention in global coordinates (k-blocks max(0,qc-8)..qc per 128-row
q-block qc), unnormalized exp softmax (bounded scores, no max pass), a
transpose-by-matmul against diag(1/rowsum) that normalizes for free, PV,
and the partial O-projection.
"""
import numpy as np
import ml_dtypes

import concourse.tile as tile
from concourse import bacc, mybir
from concourse.bass_utils import run_bass_kernel_spmd
from contextlib import ExitStack

F32 = mybir.dt.float32
BF16 = mybir.dt.bfloat16
EXP = mybir.ActivationFunctionType.Exp

B, S, DIM = 2, 2048, 2048
H, KVH, HD = 16, 4, 128
HPC = H // KVH  # heads per core (4)
WINDOW = 1024
SCALE = HD ** -0.5
NDC = DIM // 128  # 16 dim chunks
NQC = S // 128    # 16 q blocks

_cache = {}


def _kblocks(qc):
    return list(range(max(0, qc - 8), qc + 1))


def _build():
    nc = bacc.Bacc("TRN2", target_bir_lowering=False, debug=False, num_devices=8)

    xt_d = nc.dram_tensor("xt", [128, 4 * NDC * 512], BF16, kind="ExternalInput")
    wq_d = nc.dram_tensor("wq", [128, 2 * NDC * 256], BF16, kind="ExternalInput")
    wkv_d = nc.dram_tensor("wkv", [128, NDC * 256], BF16, kind="ExternalInput")
    wo_d = nc.dram_tensor("wo", [128, 2 * 2 * 2048], BF16, kind="ExternalInput")
    fm_d = nc.dram_tensor("fm", [64, S], F32, kind="ExternalInput")
    fp_d = nc.dram_tensor("fp", [64, S], F32, kind="ExternalInput")
    t0_d = nc.dram_tensor("t0", [128, 128], F32, kind="ExternalInput")
    t8_d = nc.dram_tensor("t8", [128, 128], F32, kind="ExternalInput")
    id_d = nc.dram_tensor("ident", [128, 128], BF16, kind="ExternalInput")
    out_d = nc.dram_tensor("out", [S, DIM], F32, kind="ExternalOutput")

    with tile.TileContext(nc) as tc, ExitStack() as ctx:
        xp = ctx.enter_context(tc.tile_pool(name="xp", bufs=2))
        wp = ctx.enter_context(tc.tile_pool(name="wp", bufs=5))
        cp = ctx.enter_context(tc.tile_pool(name="cp", bufs=1))
        qp = ctx.enter_context(tc.tile_pool(name="qp", bufs=1))
        kp = ctx.enter_context(tc.tile_pool(name="kp", bufs=1))
        vp = ctx.enter_context(tc.tile_pool(name="vp", bufs=1))
        pp = ctx.enter_context(tc.tile_pool(name="pp", bufs=2))
        ptp = ctx.enter_context(tc.tile_pool(name="ptp", bufs=2))
        dgp = ctx.enter_context(tc.tile_pool(name="dgp", bufs=2))
        smp = ctx.enter_context(tc.tile_pool(name="smp", bufs=8))
        ap_ = ctx.enter_context(tc.tile_pool(name="ap", bufs=1))
        op_ = ctx.enter_context(tc.tile_pool(name="op", bufs=2))
        ps = ctx.enter_context(tc.tile_pool(name="ps", bufs=5, space="PSUM"))
        sps = ctx.enter_context(tc.tile_pool(name="sps", bufs=3, space="PSUM"))

        # ---- weights for phase 1 first (prologue-critical DMA order) ----
        wkv_t = wp.tile([128, NDC, 256], BF16, tag="w")  # cols: [wk 128 | wv 128]
        nc.sync.dma_start(wkv_t[:], wkv_d[:, :])
        wq_t = None  # allocated after the first x chunk's DMAs

        q1 = qp.tile([128, HPC, S], BF16, tag="q1")
        q2 = qp.tile([64, HPC, S], BF16, tag="q2")
        k1 = kp.tile([128, S], BF16, tag="k1")
        k2 = kp.tile([64, S], BF16, tag="k2")
        v_sb = vp.tile([128, NQC, 128], BF16, tag="v")

        # ---- fused projections + attention + O-proj per column-quarter ----
        fm = fp = t0 = t8 = ident = wo_t = None
        attn = ap_.tile([128, HPC, S], BF16, tag="attn")
        for cq in range(4):
            x_q = xp.tile([128, NDC, 512], BF16, tag="x")
            for dg in range(4):
                nc.sync.dma_start(
                    x_q[:, dg * 4 : (dg + 1) * 4, :],
                    xt_d[
                        :,
                        cq * NDC * 512 + dg * 4 * 512 : cq * NDC * 512
                        + (dg + 1) * 4 * 512,
                    ],
                )
            if cq == 0:
                # wq + constants ride after the first x chunk (not prologue-critical)
                wq_t = [
                    wp.tile([128, NDC, 256], BF16, tag="w", name=f"wq{i}")
                    for i in range(2)
                ]
                for i in range(2):
                    nc.sync.dma_start(
                        wq_t[i][:], wq_d[:, i * NDC * 256 : (i + 1) * NDC * 256]
                    )
                fm = cp.tile([64, S], F32, tag="fm")
                nc.sync.dma_start(fm[:], fm_d[:, :])
                fp = cp.tile([64, S], F32, tag="fp")
                nc.sync.dma_start(fp[:], fp_d[:, :])
                t0 = cp.tile([128, 128], F32, tag="t0")
                nc.sync.dma_start(t0[:], t0_d[:, :])
                t8 = cp.tile([128, 128], F32, tag="t8")
                nc.sync.dma_start(t8[:], t8_d[:, :])
                ident = cp.tile([128, 128], BF16, tag="ident")
                nc.sync.dma_start(ident[:], id_d[:, :])
                wo_t = [
                    wp.tile([128, 2, 2048], BF16, tag="w", name=f"wo{i}")
                    for i in range(2)
                ]
                for i in range(2):
                    nc.sync.dma_start(wo_t[i][:], wo_d[:, i * 4096 : (i + 1) * 4096])
            cs = slice(cq * 512, (cq + 1) * 512)
            fmc, fpc = fm[:, cs], fp[:, cs]

            kps = ps.tile([128, 512], F32, tag="ps")
            for dc in range(NDC):
                nc.tensor.matmul(
                    kps[:],
                    wkv_t[:, dc, 0:128],
                    x_q[:, dc, :],
                    start=(dc == 0),
                    stop=(dc == NDC - 1),
                )
            nc.vector.tensor_mul(k1[0:64, cs], kps[0:64, :], fmc)
            nc.vector.tensor_mul(k1[64:128, cs], kps[0:64, :], fpc)
            nc.scalar.copy(k2[:, cs], kps[64:128, :])

            vps = ps.tile([128, 512], F32, tag="ps")
            for kb4 in range(4):
                kb = cq * 4 + kb4
                for dc in range(NDC):
                    nc.tensor.matmul(
                        vps[:, kb4 * 128 : (kb4 + 1) * 128],
                        x_q[:, dc, kb4 * 128 : (kb4 + 1) * 128],
                        wkv_t[:, dc, 128:256],
                        start=(dc == 0),
                        stop=(dc == NDC - 1),
                    )
            for kb4 in range(4):
                nc.any.tensor_copy(
                    v_sb[:, cq * 4 + kb4, :], vps[:, kb4 * 128 : (kb4 + 1) * 128]
                )

            for h in range(HPC):
                qps = ps.tile([128, 512], F32, tag="ps")
                for dc in range(NDC):
                    nc.tensor.matmul(
                        qps[:],
                        wq_t[h // 2][:, dc, (h % 2) * 128 : (h % 2 + 1) * 128],
                        x_q[:, dc, :],
                        start=(dc == 0),
                        stop=(dc == NDC - 1),
                    )
                nc.vector.tensor_mul(q1[0:64, h, cs], qps[0:64, :], fmc)
                nc.vector.tensor_mul(q1[64:128, h, cs], qps[0:64, :], fpc)
                nc.scalar.copy(q2[:, h, cs], qps[64:128, :])

            # ---- attention for q-group qg=cq + interleaved partial O-proj ----
            qg = cq
            for h in range(HPC):
                aps = ps.tile([128, 512], F32, tag="ps")
                for qc4 in range(4):
                    qc = qg * 4 + qc4
                    qb = qc * 128
                    kbs = _kblocks(qc)
                    nkb = len(kbs)
                    chunks = [kbs[i : i + 3] for i in range(0, nkb, 3)]
                    schunks = []
                    for chunk in chunks:
                        w = len(chunk) * 128
                        sp = sps.tile([128, 384], F32, tag="s")
                        lo = chunk[0] * 128
                        nc.tensor.matmul(
                            sp[:, 0:w],
                            q1[:, h, qb : qb + 128],
                            k1[:, lo : lo + w],
                            start=True,
                            stop=False,
                        )
                        nc.tensor.matmul(
                            sp[:, 0:w],
                            q2[:, h, qb : qb + 128],
                            k2[:, lo : lo + w],
                            start=False,
                            stop=True,
                        )
                        schunks.append(sp)
                    # masks: window-tail triangle on k-block qc-8, causal on qc
                    if kbs[0] == qc - 8:
                        nc.vector.tensor_add(
                            schunks[0][:, 0:128], schunks[0][:, 0:128], t0[:]
                        )
                    dpos = (nkb - 1) % 3
                    nc.vector.tensor_add(
                        schunks[-1][:, dpos * 128 : (dpos + 1) * 128],
                        schunks[-1][:, dpos * 128 : (dpos + 1) * 128],
                        t8[:],
                    )
                    # exp + row sums
                    p_sb = pp.tile([128, 1152], BF16, tag="p")
                    acc = smp.tile([128, 3], F32, tag="acc")
                    for ci, chunk in enumerate(chunks):
                        w = len(chunk) * 128
                        nc.scalar.activation(
                            p_sb[:, ci * 384 : ci * 384 + w],
                            schunks[ci][:, 0:w],
                            EXP,
                            accum_out=acc[:, ci : ci + 1],
                        )
                    sm = smp.tile([128, 1], F32, tag="sm")
                    if len(chunks) == 1:
                        nc.vector.tensor_copy(sm[:], acc[:, 0:1])
                    else:
                        nc.vector.tensor_add(sm[:], acc[:, 0:1], acc[:, 1:2])
                        if len(chunks) == 3:
                            nc.vector.tensor_add(sm[:], sm[:], acc[:, 2:3])
                    rc = smp.tile([128, 1], F32, tag="rc")
                    nc.vector.reciprocal(rc[:], sm[:])
                    dg = dgp.tile([128, 128], BF16, tag="dg")
                    nc.vector.tensor_scalar_mul(dg[:], ident[:], rc[:, 0:1])
                    # normalized transpose: PT[k,q] = P^T @ diag(1/sum)
                    pt_sb = ptp.tile([128, 1152], BF16, tag="pt")
                    for ci, chunk in enumerate(chunks):
                        w = len(chunk) * 128
                        ptps = ps.tile([128, 512], F32, tag="ps")
                        for t in range(len(chunk)):
                            nc.tensor.matmul(
                                ptps[:, t * 128 : (t + 1) * 128],
                                p_sb[:, ci * 384 + t * 128 : ci * 384 + (t + 1) * 128],
                                dg[:],
                                start=True,
                                stop=True,
                            )
                        nc.any.tensor_copy(
                            pt_sb[:, ci * 384 : ci * 384 + w], ptps[:, 0:w]
                        )
                    # PV
                    for mi, kb in enumerate(kbs):
                        ci, t = mi // 3, mi % 3
                        nc.tensor.matmul(
                            aps[:, qc4 * 128 : (qc4 + 1) * 128],
                            v_sb[:, kb, :],
                            pt_sb[:, ci * 384 + t * 128 : ci * 384 + (t + 1) * 128],
                            start=(mi == 0),
                            stop=(mi == nkb - 1),
                        )
                nc.any.tensor_copy(attn[:, h, qg * 512 : (qg + 1) * 512], aps[:])

            # partial O-projection for this q-group (overlaps next group's attn)
            for qc in range(qg * 4, (qg + 1) * 4):
                o_sb = op_.tile([128, 2048], F32, tag="o")
                for dn in range(4):
                    ops = ps.tile([128, 512], F32, tag="ps")
                    for f in range(HPC):
                        nc.tensor.matmul(
                            ops[:],
                            attn[:, f, qc * 128 : (qc + 1) * 128],
                            wo_t[f // 2][:, f % 2, dn * 512 : (dn + 1) * 512],
                            start=(f == 0),
                            stop=(f == HPC - 1),
                        )
                    nc.any.tensor_copy(o_sb[:, dn * 512 : (dn + 1) * 512], ops[:])
                nc.sync.dma_start(out_d[qc * 128 : (qc + 1) * 128, :], o_sb[:])

    nc.compile()
    return nc


def _prep_core(inputs, c):
    x = inputs["x"]
    cos, sin = np.asarray(inputs["cos"]), np.asarray(inputs["sin"])
    mask = np.asarray(inputs["mask"])
    wq = np.asarray(inputs["wq"], dtype=np.float32)
    wk = np.asarray(inputs["wk"], dtype=np.float32)
    wv = np.asarray(inputs["wv"], dtype=np.float32)
    wo = np.asarray(inputs["wo"], dtype=np.float32)
    bf = ml_dtypes.bfloat16
    b, g = c // 4, c % 4

    # x[b] transposed -> [128p, cq, dc, 512]
    xt = np.asarray(x[b], dtype=np.float32).T  # [dim, S]
    xt = xt.reshape(NDC, 128, 4, 512).transpose(1, 2, 0, 3)
    xt = np.ascontiguousarray(xt).reshape(128, 4 * NDC * 512).astype(bf)

    # wq slice for heads 4g..4g+3 (SCALE folded), [p, hpair, dc, 256]
    wqs = (wq[:, g * 512 : (g + 1) * 512] * SCALE).reshape(NDC, 128, 2, 256)
    wqs = np.ascontiguousarray(wqs.transpose(1, 2, 0, 3)).reshape(128, 2 * NDC * 256)
    # wk|wv slice for kv head g: [p, dc, 256] with cols [wk 128 | wv 128]
    wkv = np.concatenate(
        [wk[:, g * 128 : (g + 1) * 128], wv[:, g * 128 : (g + 1) * 128]], axis=1
    )
    wkv = np.ascontiguousarray(wkv.reshape(NDC, 128, 256).transpose(1, 0, 2)).reshape(
        128, NDC * 256
    )
    # wo rows for this core's heads: [p, f2(2 within pair), ...] tiles [128,2,2048]
    wos = wo[g * 512 : (g + 1) * 512].reshape(2, 2, 128, 2048).transpose(2, 0, 1, 3)
    wos = np.ascontiguousarray(wos).reshape(128, 2 * 2 * 2048)

    fm = np.ascontiguousarray((cos - sin).T, dtype=np.float32)
    fp_ = np.ascontiguousarray((cos + sin).T, dtype=np.float32)
    t0 = np.ascontiguousarray(mask[WINDOW : WINDOW + 128, 0:128], dtype=np.float32)
    t8 = np.ascontiguousarray(mask[0:128, 0:128], dtype=np.float32)

    return {
        "xt": xt, "wq": wqs.astype(bf), "wkv": wkv.astype(bf), "wo": wos.astype(bf),
        "fm": fm, "fp": fp_, "t0": t0, "t8": t8,
        "ident": np.eye(128, dtype=np.float32).astype(bf),
    }


def kernel(**inputs) -> np.ndarray:
    if "nc" not in _cache:
        _cache["nc"] = _build()
    nc = _cache["nc"]
    in_maps = [_prep_core(inputs, c) for c in range(8)]
    res = run_bass_kernel_spmd(nc, in_maps, core_ids=list(range(8)))
    out = np.zeros((B, S, DIM), dtype=np.float32)
    for c in range(8):
        out[c // 4] += res.results[c]["out"]
    return out


# revision 18
# speedup vs baseline: 1.6018x; 1.0886x over previous
"""Distributed Bass kernel for sliding-window GQA attention on 8 TRN2 NeuronCores.

Problem: B=2, S=2048, DIM=2048, H=16, KVH=4, HD=128, WINDOW=1024 (causal
sliding window), nonstandard RoPE producing 1.5*HD score features.

Sharding (tensor-parallel on the kv-head axis, data-parallel on batch —
no collectives): core c owns (batch, kv-group) = (c//4, c%4): its 4 q-heads
and 1 kv head over the full 2048-row sequence. wq/wk/wv are column-sharded
by kv group, wo row-sharded. Each core emits a PARTIAL output projection
(its 4 heads x its wo rows); the host sums the 4 partials per batch while
unsharding — replacing the all-reduce.

Per core: Q/K/V projections + rope scaling, block-sparse sliding-window
attention in global coordinates (k-blocks max(0,qc-8)..qc per 128-row
q-block qc), unnormalized exp softmax (bounded scores, no max pass), a
transpose-by-matmul against diag(1/rowsum) that normalizes for free, PV,
and the partial O-projection.
"""
import numpy as np
import ml_dtypes

import concourse.tile as tile
from concourse import bacc, mybir
from concourse.bass_utils import run_bass_kernel_spmd
from contextlib import ExitStack

F32 = mybir.dt.float32
BF16 = mybir.dt.bfloat16
EXP = mybir.ActivationFunctionType.Exp

B, S, DIM = 2, 2048, 2048
H, KVH, HD = 16, 4, 128
HPC = H // KVH  # heads per core (4)
WINDOW = 1024
SCALE = HD ** -0.5
NDC = DIM // 128  # 16 dim chunks
NQC = S // 128    # 16 q blocks

_cache = {}


def _kblocks(qc):
    return list(range(max(0, qc - 8), qc + 1))


def _build():
    nc = bacc.Bacc("TRN2", target_bir_lowering=False, debug=False, num_devices=8)

    xt_d = nc.dram_tensor("xt", [128, 4 * NDC * 512], BF16, kind="ExternalInput")
    wq_d = nc.dram_tensor("wq", [128, 2 * NDC * 256], BF16, kind="ExternalInput")
    wkv_d = nc.dram_tensor("wkv", [128, NDC * 256], BF16, kind="ExternalInput")
    wo_d = nc.dram_tensor("wo", [128, 2 * 2 * 2048], BF16, kind="ExternalInput")
    fm_d = nc.dram_tensor("fm", [64, S], F32, kind="ExternalInput")
    fp_d = nc.dram_tensor("fp", [64, S], F32, kind="ExternalInput")
    t0_d = nc.dram_tensor("t0", [128, 128], F32, kind="ExternalInput")
    t8_d = nc.dram_tensor("t8", [128, 128], F32, kind="ExternalInput")
    id_d = nc.dram_tensor("ident", [128, 128], BF16, kind="ExternalInput")
    out_d = nc.dram_tensor("out", [S, DIM], F32, kind="ExternalOutput")

    with tile.TileContext(nc) as tc, ExitStack() as ctx:
        xp = ctx.enter_context(tc.tile_pool(name="xp", bufs=3))
        wp = ctx.enter_context(tc.tile_pool(name="wp", bufs=3))
        cp = ctx.enter_context(tc.tile_pool(name="cp", bufs=1))
        qp = ctx.enter_context(tc.tile_pool(name="qp", bufs=1))
        kp = ctx.enter_context(tc.tile_pool(name="kp", bufs=1))
        vp = ctx.enter_context(tc.tile_pool(name="vp", bufs=1))
        pp = ctx.enter_context(tc.tile_pool(name="pp", bufs=2))
        ptp = ctx.enter_context(tc.tile_pool(name="ptp", bufs=2))
        dgp = ctx.enter_context(tc.tile_pool(name="dgp", bufs=2))
        smp = ctx.enter_context(tc.tile_pool(name="smp", bufs=8))
        ap_ = ctx.enter_context(tc.tile_pool(name="ap", bufs=1))
        op_ = ctx.enter_context(tc.tile_pool(name="op", bufs=2))
        ps = ctx.enter_context(tc.tile_pool(name="ps", bufs=5, space="PSUM"))
        sps = ctx.enter_context(tc.tile_pool(name="sps", bufs=3, space="PSUM"))

        # ---- weights for phase 1 first (prologue-critical DMA order) ----
        wkv_t = wp.tile([128, NDC, 256], BF16, tag="w")  # cols: [wk 128 | wv 128]
        for i in range(2):
            nc.sync.dma_start(
                wkv_t[:, i * 8 : (i + 1) * 8, :],
                wkv_d[:, i * 8 * 256 : (i + 1) * 8 * 256],
            )
        wq_t = None  # allocated after the first x chunk's DMAs

        q1 = qp.tile([128, HPC, S], BF16, tag="q1")
        q2 = qp.tile([64, HPC, S], BF16, tag="q2")
        k1 = kp.tile([128, S], BF16, tag="k1")
        k2 = kp.tile([64, S], BF16, tag="k2")
        v_sb = vp.tile([128, NQC, 128], BF16, tag="v")

        # ---- fused projections + attention + O-proj per column-quarter ----
        fm = fp = t0 = t8 = ident = wo_t = None
        attn = ap_.tile([128, HPC, S], BF16, tag="attn")
        for cq in range(4):
            x_q = xp.tile([128, NDC, 512], BF16, tag="x")
            for dg in range(4):
                nc.sync.dma_start(
                    x_q[:, dg * 4 : (dg + 1) * 4, :],
                    xt_d[
                        :,
                        cq * NDC * 512 + dg * 4 * 512 : cq * NDC * 512
                        + (dg + 1) * 4 * 512,
                    ],
                )
            if cq == 0:
                # wq + constants ride after the first x chunk (not prologue-critical)
                wq_t = [
                    wp.tile([128, NDC, 256], BF16, tag="w", name=f"wq{i}")
                    for i in range(2)
                ]
                for i in range(2):
                    nc.sync.dma_start(
                        wq_t[i][:], wq_d[:, i * NDC * 256 : (i + 1) * NDC * 256]
                    )
                fm = cp.tile([64, S], F32, tag="fm")
                nc.sync.dma_start(fm[:], fm_d[:, :])
                fp = cp.tile([64, S], F32, tag="fp")
                nc.sync.dma_start(fp[:], fp_d[:, :])
                t0 = cp.tile([128, 128], F32, tag="t0")
                nc.sync.dma_start(t0[:], t0_d[:, :])
                t8 = cp.tile([128, 128], F32, tag="t8")
                nc.sync.dma_start(t8[:], t8_d[:, :])
                ident = cp.tile([128, 128], BF16, tag="ident")
                nc.sync.dma_start(ident[:], id_d[:, :])
            cs = slice(cq * 512, (cq + 1) * 512)
            fmc, fpc = fm[:, cs], fp[:, cs]

            kps = ps.tile([128, 512], F32, tag="ps")
            for dc in range(NDC):
                nc.tensor.matmul(
                    kps[:],
                    wkv_t[:, dc, 0:128],
                    x_q[:, dc, :],
                    start=(dc == 0),
                    stop=(dc == NDC - 1),
                )
            nc.vector.tensor_mul(k1[0:64, cs], kps[0:64, :], fmc)
            nc.vector.tensor_mul(k1[64:128, cs], kps[0:64, :], fpc)
            nc.scalar.copy(k2[:, cs], kps[64:128, :])

            vps = ps.tile([128, 512], F32, tag="ps")
            for kb4 in range(4):
                kb = cq * 4 + kb4
                for dc in range(NDC):
                    nc.tensor.matmul(
                        vps[:, kb4 * 128 : (kb4 + 1) * 128],
                        x_q[:, dc, kb4 * 128 : (kb4 + 1) * 128],
                        wkv_t[:, dc, 128:256],
                        start=(dc == 0),
                        stop=(dc == NDC - 1),
                    )
            for kb4 in range(4):
                nc.any.tensor_copy(
                    v_sb[:, cq * 4 + kb4, :], vps[:, kb4 * 128 : (kb4 + 1) * 128]
                )

            for h in range(HPC):
                qps = ps.tile([128, 512], F32, tag="ps")
                for dc in range(NDC):
                    nc.tensor.matmul(
                        qps[:],
                        wq_t[h // 2][:, dc, (h % 2) * 128 : (h % 2 + 1) * 128],
                        x_q[:, dc, :],
                        start=(dc == 0),
                        stop=(dc == NDC - 1),
                    )
                nc.vector.tensor_mul(q1[0:64, h, cs], qps[0:64, :], fmc)
                nc.vector.tensor_mul(q1[64:128, h, cs], qps[0:64, :], fpc)
                nc.scalar.copy(q2[:, h, cs], qps[64:128, :])

        # ---- attention + per-group O-projection ----
        wo_t = [
            wp.tile([128, 2, 2048], BF16, tag="w", name=f"wo{i}") for i in range(2)
        ]
        for i in range(2):
            nc.sync.dma_start(wo_t[i][:], wo_d[:, i * 4096 : (i + 1) * 4096])
        for qg in range(4):
            for h in range(HPC):
                aps = ps.tile([128, 512], F32, tag="ps")
                for qc4 in range(4):
                    qc = qg * 4 + qc4
                    qb = qc * 128
                    kbs = _kblocks(qc)
                    nkb = len(kbs)
                    chunks = [kbs[i : i + 3] for i in range(0, nkb, 3)]
                    schunks = []
                    for chunk in chunks:
                        w = len(chunk) * 128
                        sp = sps.tile([128, 384], F32, tag="s")
                        lo = chunk[0] * 128
                        nc.tensor.matmul(
                            sp[:, 0:w],
                            q1[:, h, qb : qb + 128],
                            k1[:, lo : lo + w],
                            start=True,
                            stop=False,
                        )
                        nc.tensor.matmul(
                            sp[:, 0:w],
                            q2[:, h, qb : qb + 128],
                            k2[:, lo : lo + w],
                            start=False,
                            stop=True,
                        )
                        schunks.append(sp)
                    # masks: window-tail triangle on k-block qc-8, causal on qc
                    if kbs[0] == qc - 8:
                        nc.vector.tensor_add(
                            schunks[0][:, 0:128], schunks[0][:, 0:128], t0[:]
                        )
                    dpos = (nkb - 1) % 3
                    nc.vector.tensor_add(
                        schunks[-1][:, dpos * 128 : (dpos + 1) * 128],
                        schunks[-1][:, dpos * 128 : (dpos + 1) * 128],
                        t8[:],
                    )
                    # exp + row sums
                    p_sb = pp.tile([128, 1152], BF16, tag="p")
                    acc = smp.tile([128, 3], F32, tag="acc")
                    for ci, chunk in enumerate(chunks):
                        w = len(chunk) * 128
                        nc.scalar.activation(
                            p_sb[:, ci * 384 : ci * 384 + w],
                            schunks[ci][:, 0:w],
                            EXP,
                            accum_out=acc[:, ci : ci + 1],
                        )
                    sm = smp.tile([128, 1], F32, tag="sm")
                    if len(chunks) == 1:
                        nc.vector.tensor_copy(sm[:], acc[:, 0:1])
                    else:
                        nc.vector.tensor_add(sm[:], acc[:, 0:1], acc[:, 1:2])
                        if len(chunks) == 3:
                            nc.vector.tensor_add(sm[:], sm[:], acc[:, 2:3])
                    rc = smp.tile([128, 1], F32, tag="rc")
                    nc.vector.reciprocal(rc[:], sm[:])
                    dg = dgp.tile([128, 128], BF16, tag="dg")
                    nc.vector.tensor_scalar_mul(dg[:], ident[:], rc[:, 0:1])
                    # normalized transpose: PT[k,q] = P^T @ diag(1/sum)
                    pt_sb = ptp.tile([128, 1152], BF16, tag="pt")
                    for ci, chunk in enumerate(chunks):
                        w = len(chunk) * 128
                        ptps = ps.tile([128, 512], F32, tag="ps")
                        for t in range(len(chunk)):
                            nc.tensor.matmul(
                                ptps[:, t * 128 : (t + 1) * 128],
                                p_sb[:, ci * 384 + t * 128 : ci * 384 + (t + 1) * 128],
                                dg[:],
                                start=True,
                                stop=True,
                            )
                        nc.any.tensor_copy(
                            pt_sb[:, ci * 384 : ci * 384 + w], ptps[:, 0:w]
                        )
                    # PV
                    for mi, kb in enumerate(kbs):
                        ci, t = mi // 3, mi % 3
                        nc.tensor.matmul(
                            aps[:, qc4 * 128 : (qc4 + 1) * 128],
                            v_sb[:, kb, :],
                            pt_sb[:, ci * 384 + t * 128 : ci * 384 + (t + 1) * 128],
                            start=(mi == 0),
                            stop=(mi == nkb - 1),
                        )
                nc.any.tensor_copy(attn[:, h, qg * 512 : (qg + 1) * 512], aps[:])

            # partial O-projection for this q-group (overlaps next group's attn)
            for qc in range(qg * 4, (qg + 1) * 4):
                o_sb = op_.tile([128, 2048], F32, tag="o")
                for dn in range(4):
                    ops = ps.tile([128, 512], F32, tag="ps")
                    for f in range(HPC):
                        nc.tensor.matmul(
                            ops[:],
                            attn[:, f, qc * 128 : (qc + 1) * 128],
                            wo_t[f // 2][:, f % 2, dn * 512 : (dn + 1) * 512],
                            start=(f == 0),
                            stop=(f == HPC - 1),
                        )
                    nc.any.tensor_copy(o_sb[:, dn * 512 : (dn + 1) * 512], ops[:])
                nc.sync.dma_start(out_d[qc * 128 : (qc + 1) * 128, :], o_sb[:])

    nc.compile()
    return nc


def _prep_core(inputs, c):
    x = inputs["x"]
    cos, sin = np.asarray(inputs["cos"]), np.asarray(inputs["sin"])
    mask = np.asarray(inputs["mask"])
    wq = np.asarray(inputs["wq"], dtype=np.float32)
    wk = np.asarray(inputs["wk"], dtype=np.float32)
    wv = np.asarray(inputs["wv"], dtype=np.float32)
    wo = np.asarray(inputs["wo"], dtype=np.float32)
    bf = ml_dtypes.bfloat16
    b, g = c // 4, c % 4

    # x[b] transposed -> [128p, cq, dc, 512]
    xt = np.asarray(x[b], dtype=np.float32).T  # [dim, S]
    xt = xt.reshape(NDC, 128, 4, 512).transpose(1, 2, 0, 3)
    xt = np.ascontiguousarray(xt).reshape(128, 4 * NDC * 512).astype(bf)

    # wq slice for heads 4g..4g+3 (SCALE folded), [p, hpair, dc, 256]
    wqs = (wq[:, g * 512 : (g + 1) * 512] * SCALE).reshape(NDC, 128, 2, 256)
    wqs = np.ascontiguousarray(wqs.transpose(1, 2, 0, 3)).reshape(128, 2 * NDC * 256)
    # wk|wv slice for kv head g: [p, dc, 256] with cols [wk 128 | wv 128]
    wkv = np.concatenate(
        [wk[:, g * 128 : (g + 1) * 128], wv[:, g * 128 : (g + 1) * 128]], axis=1
    )
    wkv = np.ascontiguousarray(wkv.reshape(NDC, 128, 256).transpose(1, 0, 2)).reshape(
        128, NDC * 256
    )
    # wo rows for this core's heads: [p, f2(2 within pair), ...] tiles [128,2,2048]
    wos = wo[g * 512 : (g + 1) * 512].reshape(2, 2, 128, 2048).transpose(2, 0, 1, 3)
    wos = np.ascontiguousarray(wos).reshape(128, 2 * 2 * 2048)

    fm = np.ascontiguousarray((cos - sin).T, dtype=np.float32)
    fp_ = np.ascontiguousarray((cos + sin).T, dtype=np.float32)
    t0 = np.ascontiguousarray(mask[WINDOW : WINDOW + 128, 0:128], dtype=np.float32)
    t8 = np.ascontiguousarray(mask[0:128, 0:128], dtype=np.float32)

    return {
        "xt": xt, "wq": wqs.astype(bf), "wkv": wkv.astype(bf), "wo": wos.astype(bf),
        "fm": fm, "fp": fp_, "t0": t0, "t8": t8,
        "ident": np.eye(128, dtype=np.float32).astype(bf),
    }


def kernel(**inputs) -> np.ndarray:
    if "nc" not in _cache:
        _cache["nc"] = _build()
    nc = _cache["nc"]
    in_maps = [_prep_core(inputs, c) for c in range(8)]
    res = run_bass_kernel_spmd(nc, in_maps, core_ids=list(range(8)))
    out = np.zeros((B, S, DIM), dtype=np.float32)
    for c in range(8):
        out[c // 4] += res.results[c]["out"]
    return out


# revision 19
# speedup vs baseline: 1.6054x; 1.0023x over previous
"""Distributed Bass kernel for sliding-window GQA attention on 8 TRN2 NeuronCores.

Problem: B=2, S=2048, DIM=2048, H=16, KVH=4, HD=128, WINDOW=1024 (causal
sliding window), nonstandard RoPE producing 1.5*HD score features.

Sharding (tensor-parallel on the kv-head axis, data-parallel on batch —
no collectives): core c owns (batch, kv-group) = (c//4, c%4): its 4 q-heads
and 1 kv head over the full 2048-row sequence. wq/wk/wv are column-sharded
by kv group, wo row-sharded. Each core emits a PARTIAL output projection
(its 4 heads x its wo rows); the host sums the 4 partials per batch while
unsharding — replacing the all-reduce.

Per core: Q/K/V projections + rope scaling, block-sparse sliding-window
attention in global coordinates (k-blocks max(0,qc-8)..qc per 128-row
q-block qc), unnormalized exp softmax (bounded scores, no max pass), a
transpose-by-matmul against diag(1/rowsum) that normalizes for free, PV,
and the partial O-projection.
"""
import numpy as np
import ml_dtypes

import concourse.tile as tile
from concourse import bacc, mybir
from concourse.bass_utils import run_bass_kernel_spmd
from contextlib import ExitStack

F32 = mybir.dt.float32
BF16 = mybir.dt.bfloat16
EXP = mybir.ActivationFunctionType.Exp

B, S, DIM = 2, 2048, 2048
H, KVH, HD = 16, 4, 128
HPC = H // KVH  # heads per core (4)
WINDOW = 1024
SCALE = HD ** -0.5
NDC = DIM // 128  # 16 dim chunks
NQC = S // 128    # 16 q blocks

_cache = {}


def _kblocks(qc):
    return list(range(max(0, qc - 8), qc + 1))


def _build():
    nc = bacc.Bacc("TRN2", target_bir_lowering=False, debug=False, num_devices=8)

    xt_d = nc.dram_tensor("xt", [128, 4 * NDC * 512], BF16, kind="ExternalInput")
    wq_d = nc.dram_tensor("wq", [128, 2 * NDC * 256], BF16, kind="ExternalInput")
    wkv_d = nc.dram_tensor("wkv", [128, NDC * 256], BF16, kind="ExternalInput")
    wo_d = nc.dram_tensor("wo", [128, 2 * 2 * 2048], BF16, kind="ExternalInput")
    fm_d = nc.dram_tensor("fm", [64, S], F32, kind="ExternalInput")
    fp_d = nc.dram_tensor("fp", [64, S], F32, kind="ExternalInput")
    t0_d = nc.dram_tensor("t0", [128, 128], F32, kind="ExternalInput")
    t8_d = nc.dram_tensor("t8", [128, 128], F32, kind="ExternalInput")
    id_d = nc.dram_tensor("ident", [128, 128], BF16, kind="ExternalInput")
    out_d = nc.dram_tensor("out", [S, DIM], F32, kind="ExternalOutput")

    with tile.TileContext(nc) as tc, ExitStack() as ctx:
        xp = ctx.enter_context(tc.tile_pool(name="xp", bufs=3))
        wp = ctx.enter_context(tc.tile_pool(name="wp", bufs=3))
        cp = ctx.enter_context(tc.tile_pool(name="cp", bufs=1))
        qp = ctx.enter_context(tc.tile_pool(name="qp", bufs=1))
        kp = ctx.enter_context(tc.tile_pool(name="kp", bufs=1))
        vp = ctx.enter_context(tc.tile_pool(name="vp", bufs=1))
        pp = ctx.enter_context(tc.tile_pool(name="pp", bufs=2))
        ptp = ctx.enter_context(tc.tile_pool(name="ptp", bufs=2))
        dgp = ctx.enter_context(tc.tile_pool(name="dgp", bufs=2))
        smp = ctx.enter_context(tc.tile_pool(name="smp", bufs=8))
        ap_ = ctx.enter_context(tc.tile_pool(name="ap", bufs=1))
        op_ = ctx.enter_context(tc.tile_pool(name="op", bufs=2))
        ps = ctx.enter_context(tc.tile_pool(name="ps", bufs=5, space="PSUM"))
        sps = ctx.enter_context(tc.tile_pool(name="sps", bufs=3, space="PSUM"))

        # ---- weights for phase 1 first (prologue-critical DMA order) ----
        wkv_t = wp.tile([128, NDC, 256], BF16, tag="w")  # cols: [wk 128 | wv 128]
        for i in range(2):
            nc.sync.dma_start(
                wkv_t[:, i * 8 : (i + 1) * 8, :],
                wkv_d[:, i * 8 * 256 : (i + 1) * 8 * 256],
            )
        wq_t = None  # allocated after the first x chunk's DMAs

        q1 = qp.tile([128, HPC, S], BF16, tag="q1")
        q2 = qp.tile([64, HPC, S], BF16, tag="q2")
        k1 = kp.tile([128, S], BF16, tag="k1")
        k2 = kp.tile([64, S], BF16, tag="k2")
        v_sb = vp.tile([128, NQC, 128], BF16, tag="v")

        # ---- fused projections + attention + O-proj per column-quarter ----
        fm = fp = t0 = t8 = ident = wo_t = None
        attn = ap_.tile([128, HPC, S], BF16, tag="attn")
        for cq in range(4):
            x_q = xp.tile([128, NDC, 512], BF16, tag="x")
            ndg = 8 if cq == 0 else 4
            w_dg = NDC // ndg
            for dg in range(ndg):
                nc.sync.dma_start(
                    x_q[:, dg * w_dg : (dg + 1) * w_dg, :],
                    xt_d[
                        :,
                        cq * NDC * 512 + dg * w_dg * 512 : cq * NDC * 512
                        + (dg + 1) * w_dg * 512,
                    ],
                )
            if cq == 0:
                # wq + constants ride after the first x chunk (not prologue-critical)
                wq_t = [
                    wp.tile([128, NDC, 256], BF16, tag="w", name=f"wq{i}")
                    for i in range(2)
                ]
                for i in range(2):
                    nc.sync.dma_start(
                        wq_t[i][:], wq_d[:, i * NDC * 256 : (i + 1) * NDC * 256]
                    )
                fm = cp.tile([64, S], F32, tag="fm")
                nc.sync.dma_start(fm[:], fm_d[:, :])
                fp = cp.tile([64, S], F32, tag="fp")
                nc.sync.dma_start(fp[:], fp_d[:, :])
                t0 = cp.tile([128, 128], F32, tag="t0")
                nc.sync.dma_start(t0[:], t0_d[:, :])
                t8 = cp.tile([128, 128], F32, tag="t8")
                nc.sync.dma_start(t8[:], t8_d[:, :])
                ident = cp.tile([128, 128], BF16, tag="ident")
                nc.sync.dma_start(ident[:], id_d[:, :])
            cs = slice(cq * 512, (cq + 1) * 512)
            fmc, fpc = fm[:, cs], fp[:, cs]

            kps = ps.tile([128, 512], F32, tag="ps")
            for dc in range(NDC):
                nc.tensor.matmul(
                    kps[:],
                    wkv_t[:, dc, 0:128],
                    x_q[:, dc, :],
                    start=(dc == 0),
                    stop=(dc == NDC - 1),
                )
            nc.vector.tensor_mul(k1[0:64, cs], kps[0:64, :], fmc)
            nc.vector.tensor_mul(k1[64:128, cs], kps[0:64, :], fpc)
            nc.scalar.copy(k2[:, cs], kps[64:128, :])

            vps = ps.tile([128, 512], F32, tag="ps")
            for kb4 in range(4):
                kb = cq * 4 + kb4
                for dc in range(NDC):
                    nc.tensor.matmul(
                        vps[:, kb4 * 128 : (kb4 + 1) * 128],
                        x_q[:, dc, kb4 * 128 : (kb4 + 1) * 128],
                        wkv_t[:, dc, 128:256],
                        start=(dc == 0),
                        stop=(dc == NDC - 1),
                    )
            for kb4 in range(4):
                nc.any.tensor_copy(
                    v_sb[:, cq * 4 + kb4, :], vps[:, kb4 * 128 : (kb4 + 1) * 128]
                )

            for h in range(HPC):
                qps = ps.tile([128, 512], F32, tag="ps")
                for dc in range(NDC):
                    nc.tensor.matmul(
                        qps[:],
                        wq_t[h // 2][:, dc, (h % 2) * 128 : (h % 2 + 1) * 128],
                        x_q[:, dc, :],
                        start=(dc == 0),
                        stop=(dc == NDC - 1),
                    )
                nc.vector.tensor_mul(q1[0:64, h, cs], qps[0:64, :], fmc)
                nc.vector.tensor_mul(q1[64:128, h, cs], qps[0:64, :], fpc)
                nc.scalar.copy(q2[:, h, cs], qps[64:128, :])

        # ---- attention + per-group O-projection ----
        wo_t = [
            wp.tile([128, 2, 2048], BF16, tag="w", name=f"wo{i}") for i in range(2)
        ]
        for i in range(2):
            nc.sync.dma_start(wo_t[i][:], wo_d[:, i * 4096 : (i + 1) * 4096])
        for qg in range(4):
            for h in range(HPC):
                aps = ps.tile([128, 512], F32, tag="ps")
                for qc4 in range(4):
                    qc = qg * 4 + qc4
                    qb = qc * 128
                    kbs = _kblocks(qc)
                    nkb = len(kbs)
                    chunks = [kbs[i : i + 3] for i in range(0, nkb, 3)]
                    schunks = []
                    for chunk in chunks:
                        w = len(chunk) * 128
                        sp = sps.tile([128, 384], F32, tag="s")
                        lo = chunk[0] * 128
                        nc.tensor.matmul(
                            sp[:, 0:w],
                            q1[:, h, qb : qb + 128],
                            k1[:, lo : lo + w],
                            start=True,
                            stop=False,
                        )
                        nc.tensor.matmul(
                            sp[:, 0:w],
                            q2[:, h, qb : qb + 128],
                            k2[:, lo : lo + w],
                            start=False,
                            stop=True,
                        )
                        schunks.append(sp)
                    # masks: window-tail triangle on k-block qc-8, causal on qc
                    if kbs[0] == qc - 8:
                        nc.vector.tensor_add(
                            schunks[0][:, 0:128], schunks[0][:, 0:128], t0[:]
                        )
                    dpos = (nkb - 1) % 3
                    nc.vector.tensor_add(
                        schunks[-1][:, dpos * 128 : (dpos + 1) * 128],
                        schunks[-1][:, dpos * 128 : (dpos + 1) * 128],
                        t8[:],
                    )
                    # exp + row sums
                    p_sb = pp.tile([128, 1152], BF16, tag="p")
                    acc = smp.tile([128, 3], F32, tag="acc")
                    for ci, chunk in enumerate(chunks):
                        w = len(chunk) * 128
                        nc.scalar.activation(
                            p_sb[:, ci * 384 : ci * 384 + w],
                            schunks[ci][:, 0:w],
                            EXP,
                            accum_out=acc[:, ci : ci + 1],
                        )
                    sm = smp.tile([128, 1], F32, tag="sm")
                    if len(chunks) == 1:
                        nc.vector.tensor_copy(sm[:], acc[:, 0:1])
                    else:
                        nc.vector.tensor_add(sm[:], acc[:, 0:1], acc[:, 1:2])
                        if len(chunks) == 3:
                            nc.vector.tensor_add(sm[:], sm[:], acc[:, 2:3])
                    rc = smp.tile([128, 1], F32, tag="rc")
                    nc.vector.reciprocal(rc[:], sm[:])
                    dg = dgp.tile([128, 128], BF16, tag="dg")
                    nc.vector.tensor_scalar_mul(dg[:], ident[:], rc[:, 0:1])
                    # normalized transpose: PT[k,q] = P^T @ diag(1/sum)
                    pt_sb = ptp.tile([128, 1152], BF16, tag="pt")
                    for ci, chunk in enumerate(chunks):
                        w = len(chunk) * 128
                        ptps = ps.tile([128, 512], F32, tag="ps")
                        for t in range(len(chunk)):
                            nc.tensor.matmul(
                                ptps[:, t * 128 : (t + 1) * 128],
                                p_sb[:, ci * 384 + t * 128 : ci * 384 + (t + 1) * 128],
                                dg[:],
                                start=True,
                                stop=True,
                            )
                        nc.any.tensor_copy(
                            pt_sb[:, ci * 384 : ci * 384 + w], ptps[:, 0:w]
                        )
                    # PV
                    for mi, kb in enumerate(kbs):
                        ci, t = mi // 3, mi % 3
                        nc.tensor.matmul(
                            aps[:, qc4 * 128 : (qc4 + 1) * 128],
                            v_sb[:, kb, :],
                            pt_sb[:, ci * 384 + t * 128 : ci * 384 + (t + 1) * 128],
                            start=(mi == 0),
                            stop=(mi == nkb - 1),
                        )
                nc.any.tensor_copy(attn[:, h, qg * 512 : (qg + 1) * 512], aps[:])

            # partial O-projection for this q-group (overlaps next group's attn)
            for qc in range(qg * 4, (qg + 1) * 4):
                o_sb = op_.tile([128, 2048], F32, tag="o")
                for dn in range(4):
                    ops = ps.tile([128, 512], F32, tag="ps")
                    for f in range(HPC):
                        nc.tensor.matmul(
                            ops[:],
                            attn[:, f, qc * 128 : (qc + 1) * 128],
                            wo_t[f // 2][:, f % 2, dn * 512 : (dn + 1) * 512],
                            start=(f == 0),
                            stop=(f == HPC - 1),
                        )
                    nc.any.tensor_copy(o_sb[:, dn * 512 : (dn + 1) * 512], ops[:])
                    nc.sync.dma_start(
                        out_d[qc * 128 : (qc + 1) * 128, dn * 512 : (dn + 1) * 512],
                        o_sb[:, dn * 512 : (dn + 1) * 512],
                    )

    nc.compile()
    return nc


def _prep_core(inputs, c):
    x = inputs["x"]
    cos, sin = np.asarray(inputs["cos"]), np.asarray(inputs["sin"])
    mask = np.asarray(inputs["mask"])
    wq = np.asarray(inputs["wq"], dtype=np.float32)
    wk = np.asarray(inputs["wk"], dtype=np.float32)
    wv = np.asarray(inputs["wv"], dtype=np.float32)
    wo = np.asarray(inputs["wo"], dtype=np.float32)
    bf = ml_dtypes.bfloat16
    b, g = c // 4, c % 4

    # x[b] transposed -> [128p, cq, dc, 512]
    xt = np.asarray(x[b], dtype=np.float32).T  # [dim, S]
    xt = xt.reshape(NDC, 128, 4, 512).transpose(1, 2, 0, 3)
    xt = np.ascontiguousarray(xt).reshape(128, 4 * NDC * 512).astype(bf)

    # wq slice for heads 4g..4g+3 (SCALE folded), [p, hpair, dc, 256]
    wqs = (wq[:, g * 512 : (g + 1) * 512] * SCALE).reshape(NDC, 128, 2, 256)
    wqs = np.ascontiguousarray(wqs.transpose(1, 2, 0, 3)).reshape(128, 2 * NDC * 256)
    # wk|wv slice for kv head g: [p, dc, 256] with cols [wk 128 | wv 128]
    wkv = np.concatenate(
        [wk[:, g * 128 : (g + 1) * 128], wv[:, g * 128 : (g + 1) * 128]], axis=1
    )
    wkv = np.ascontiguousarray(wkv.reshape(NDC, 128, 256).transpose(1, 0, 2)).reshape(
        128, NDC * 256
    )
    # wo rows for this core's heads: [p, f2(2 within pair), ...] tiles [128,2,2048]
    wos = wo[g * 512 : (g + 1) * 512].reshape(2, 2, 128, 2048).transpose(2, 0, 1, 3)
    wos = np.ascontiguousarray(wos).reshape(128, 2 * 2 * 2048)

    fm = np.ascontiguousarray((cos - sin).T, dtype=np.float32)
    fp_ = np.ascontiguousarray((cos + sin).T, dtype=np.float32)
    t0 = np.ascontiguousarray(mask[WINDOW : WINDOW + 128, 0:128], dtype=np.float32)
    t8 = np.ascontiguousarray(mask[0:128, 0:128], dtype=np.float32)

    return {
        "xt": xt, "wq": wqs.astype(bf), "wkv": wkv.astype(bf), "wo": wos.astype(bf),
        "fm": fm, "fp": fp_, "t0": t0, "t8": t8,
        "ident": np.eye(128, dtype=np.float32).astype(bf),
    }


def kernel(**inputs) -> np.ndarray:
    if "nc" not in _cache:
        _cache["nc"] = _build()
    nc = _cache["nc"]
    in_maps = [_prep_core(inputs, c) for c in range(8)]
    res = run_bass_kernel_spmd(nc, in_maps, core_ids=list(range(8)))
    out = np.zeros((B, S, DIM), dtype=np.float32)
    for c in range(8):
        out[c // 4] += res.results[c]["out"]
    return out


# revision 20
# speedup vs baseline: 1.6236x; 1.0113x over previous
"""Distributed Bass kernel for sliding-window GQA attention on 8 TRN2 NeuronCores.

Problem: B=2, S=2048, DIM=2048, H=16, KVH=4, HD=128, WINDOW=1024 (causal
sliding window), nonstandard RoPE producing 1.5*HD score features.

Sharding (tensor-parallel on the kv-head axis, data-parallel on batch —
no collectives): core c owns (batch, kv-group) = (c//4, c%4): its 4 q-heads
and 1 kv head over the full 2048-row sequence. wq/wk/wv are column-sharded
by kv group, wo row-sharded. Each core emits a PARTIAL output projection
(its 4 heads x its wo rows); the host sums the 4 partials per batch while
unsharding — replacing the all-reduce.

Per core: Q/K/V projections + rope scaling, block-sparse sliding-window
attention in global coordinates (k-blocks max(0,qc-8)..qc per 128-row
q-block qc), unnormalized exp softmax (bounded scores, no max pass), a
transpose-by-matmul against diag(1/rowsum) that normalizes for free, PV,
and the partial O-projection.
"""
import numpy as np
import ml_dtypes

import concourse.tile as tile
from concourse import bacc, mybir
from concourse.bass_utils import run_bass_kernel_spmd
from contextlib import ExitStack

F32 = mybir.dt.float32
BF16 = mybir.dt.bfloat16
EXP = mybir.ActivationFunctionType.Exp

B, S, DIM = 2, 2048, 2048
H, KVH, HD = 16, 4, 128
HPC = H // KVH  # heads per core (4)
WINDOW = 1024
SCALE = HD ** -0.5
NDC = DIM // 128  # 16 dim chunks
NQC = S // 128    # 16 q blocks

_cache = {}


def _kblocks(qc):
    return list(range(max(0, qc - 8), qc + 1))


def _build():
    nc = bacc.Bacc("TRN2", target_bir_lowering=False, debug=False, num_devices=8)

    xt_d = nc.dram_tensor("xt", [128, 4 * NDC * 512], BF16, kind="ExternalInput")
    wq_d = nc.dram_tensor("wq", [128, 2 * NDC * 256], BF16, kind="ExternalInput")
    wkv_d = nc.dram_tensor("wkv", [128, NDC * 256], BF16, kind="ExternalInput")
    wo_d = nc.dram_tensor("wo", [128, 2 * 2 * 2048], BF16, kind="ExternalInput")
    fm_d = nc.dram_tensor("fm", [64, S], F32, kind="ExternalInput")
    fp_d = nc.dram_tensor("fp", [64, S], F32, kind="ExternalInput")
    t0_d = nc.dram_tensor("t0", [128, 128], F32, kind="ExternalInput")
    t8_d = nc.dram_tensor("t8", [128, 128], F32, kind="ExternalInput")
    id_d = nc.dram_tensor("ident", [128, 128], BF16, kind="ExternalInput")
    out_d = nc.dram_tensor("out", [S, DIM], F32, kind="ExternalOutput")

    with tile.TileContext(nc) as tc, ExitStack() as ctx:
        xp = ctx.enter_context(tc.tile_pool(name="xp", bufs=3))
        wp = ctx.enter_context(tc.tile_pool(name="wp", bufs=3))
        cp = ctx.enter_context(tc.tile_pool(name="cp", bufs=1))
        qp = ctx.enter_context(tc.tile_pool(name="qp", bufs=1))
        kp = ctx.enter_context(tc.tile_pool(name="kp", bufs=1))
        vp = ctx.enter_context(tc.tile_pool(name="vp", bufs=1))
        pp = ctx.enter_context(tc.tile_pool(name="pp", bufs=2))
        ptp = ctx.enter_context(tc.tile_pool(name="ptp", bufs=2))
        dgp = ctx.enter_context(tc.tile_pool(name="dgp", bufs=2))
        smp = ctx.enter_context(tc.tile_pool(name="smp", bufs=8))
        ap_ = ctx.enter_context(tc.tile_pool(name="ap", bufs=1))
        op_ = ctx.enter_context(tc.tile_pool(name="op", bufs=2))
        ps = ctx.enter_context(tc.tile_pool(name="ps", bufs=5, space="PSUM"))
        sps = ctx.enter_context(tc.tile_pool(name="sps", bufs=3, space="PSUM"))

        # ---- weights for phase 1 first (prologue-critical DMA order) ----
        wkv_t = wp.tile([128, NDC, 256], BF16, tag="w")  # cols: [wk 128 | wv 128]
        for i in range(2):
            nc.sync.dma_start(
                wkv_t[:, i * 8 : (i + 1) * 8, :],
                wkv_d[:, i * 8 * 256 : (i + 1) * 8 * 256],
            )
        wq_t = None  # allocated after the first x chunk's DMAs

        q1 = qp.tile([128, HPC, S], BF16, tag="q1")
        q2 = qp.tile([64, HPC, S], BF16, tag="q2")
        k1 = kp.tile([128, S], BF16, tag="k1")
        k2 = kp.tile([64, S], BF16, tag="k2")
        v_sb = vp.tile([128, NQC, 128], BF16, tag="v")

        # ---- fused projections + attention + O-proj per column-quarter ----
        fm = fp = t0 = t8 = ident = wo_t = None
        attn = ap_.tile([128, HPC, S], BF16, tag="attn")
        for cq in range(4):
            x_q = xp.tile([128, NDC, 512], BF16, tag="x")
            ndg = 8 if cq == 0 else 4
            w_dg = NDC // ndg
            for dg in range(ndg):
                nc.sync.dma_start(
                    x_q[:, dg * w_dg : (dg + 1) * w_dg, :],
                    xt_d[
                        :,
                        cq * NDC * 512 + dg * w_dg * 512 : cq * NDC * 512
                        + (dg + 1) * w_dg * 512,
                    ],
                )
            if cq == 0:
                # wq + constants ride after the first x chunk (not prologue-critical)
                wq_t = [
                    wp.tile([128, NDC, 256], BF16, tag="w", name=f"wq{i}")
                    for i in range(2)
                ]
                for i in range(2):
                    nc.sync.dma_start(
                        wq_t[i][:], wq_d[:, i * NDC * 256 : (i + 1) * NDC * 256]
                    )
                fm = cp.tile([64, S], F32, tag="fm")
                nc.sync.dma_start(fm[:], fm_d[:, :])
                fp = cp.tile([64, S], F32, tag="fp")
                nc.sync.dma_start(fp[:], fp_d[:, :])
                t0 = cp.tile([128, 128], F32, tag="t0")
                nc.sync.dma_start(t0[:], t0_d[:, :])
                t8 = cp.tile([128, 128], F32, tag="t8")
                nc.sync.dma_start(t8[:], t8_d[:, :])
                ident = cp.tile([128, 128], BF16, tag="ident")
                nc.sync.dma_start(ident[:], id_d[:, :])
            cs = slice(cq * 512, (cq + 1) * 512)
            fmc, fpc = fm[:, cs], fp[:, cs]

            kps = ps.tile([128, 512], F32, tag="ps")
            for dc in range(NDC):
                nc.tensor.matmul(
                    kps[:],
                    wkv_t[:, dc, 0:128],
                    x_q[:, dc, :],
                    start=(dc == 0),
                    stop=(dc == NDC - 1),
                )
            nc.vector.tensor_mul(k1[0:64, cs], kps[0:64, :], fmc)
            nc.vector.tensor_mul(k1[64:128, cs], kps[0:64, :], fpc)
            nc.scalar.copy(k2[:, cs], kps[64:128, :])

            vps = ps.tile([128, 512], F32, tag="ps")
            for kb4 in range(4):
                kb = cq * 4 + kb4
                for dc in range(NDC):
                    nc.tensor.matmul(
                        vps[:, kb4 * 128 : (kb4 + 1) * 128],
                        x_q[:, dc, kb4 * 128 : (kb4 + 1) * 128],
                        wkv_t[:, dc, 128:256],
                        start=(dc == 0),
                        stop=(dc == NDC - 1),
                    )
            for kb4 in range(4):
                nc.any.tensor_copy(
                    v_sb[:, cq * 4 + kb4, :], vps[:, kb4 * 128 : (kb4 + 1) * 128]
                )

            for h in range(HPC):
                qps = ps.tile([128, 512], F32, tag="ps")
                for dc in range(NDC):
                    nc.tensor.matmul(
                        qps[:],
                        wq_t[h // 2][:, dc, (h % 2) * 128 : (h % 2 + 1) * 128],
                        x_q[:, dc, :],
                        start=(dc == 0),
                        stop=(dc == NDC - 1),
                    )
                nc.vector.tensor_mul(q1[0:64, h, cs], qps[0:64, :], fmc)
                nc.vector.tensor_mul(q1[64:128, h, cs], qps[0:64, :], fpc)
                nc.scalar.copy(q2[:, h, cs], qps[64:128, :])

        # ---- attention + per-group O-projection ----
        wo_t = [
            wp.tile([128, 2, 2048], BF16, tag="w", name=f"wo{i}") for i in range(2)
        ]
        for i in range(2):
            nc.sync.dma_start(wo_t[i][:], wo_d[:, i * 4096 : (i + 1) * 4096])
        for qc in range(NQC):
            qb = qc * 128
            for h in range(HPC):
                if True:
                    aps = ps.tile([128, 128], F32, tag="ps")
                    kbs = _kblocks(qc)
                    nkb = len(kbs)
                    chunks = [kbs[i : i + 3] for i in range(0, nkb, 3)]
                    schunks = []
                    for chunk in chunks:
                        w = len(chunk) * 128
                        sp = sps.tile([128, 384], F32, tag="s")
                        lo = chunk[0] * 128
                        nc.tensor.matmul(
                            sp[:, 0:w],
                            q1[:, h, qb : qb + 128],
                            k1[:, lo : lo + w],
                            start=True,
                            stop=False,
                        )
                        nc.tensor.matmul(
                            sp[:, 0:w],
                            q2[:, h, qb : qb + 128],
                            k2[:, lo : lo + w],
                            start=False,
                            stop=True,
                        )
                        schunks.append(sp)
                    # masks: window-tail triangle on k-block qc-8, causal on qc
                    if kbs[0] == qc - 8:
                        nc.vector.tensor_add(
                            schunks[0][:, 0:128], schunks[0][:, 0:128], t0[:]
                        )
                    dpos = (nkb - 1) % 3
                    nc.vector.tensor_add(
                        schunks[-1][:, dpos * 128 : (dpos + 1) * 128],
                        schunks[-1][:, dpos * 128 : (dpos + 1) * 128],
                        t8[:],
                    )
                    # exp + row sums
                    p_sb = pp.tile([128, 1152], BF16, tag="p")
                    acc = smp.tile([128, 3], F32, tag="acc")
                    for ci, chunk in enumerate(chunks):
                        w = len(chunk) * 128
                        nc.scalar.activation(
                            p_sb[:, ci * 384 : ci * 384 + w],
                            schunks[ci][:, 0:w],
                            EXP,
                            accum_out=acc[:, ci : ci + 1],
                        )
                    sm = smp.tile([128, 1], F32, tag="sm")
                    if len(chunks) == 1:
                        nc.vector.tensor_copy(sm[:], acc[:, 0:1])
                    else:
                        nc.vector.tensor_add(sm[:], acc[:, 0:1], acc[:, 1:2])
                        if len(chunks) == 3:
                            nc.vector.tensor_add(sm[:], sm[:], acc[:, 2:3])
                    rc = smp.tile([128, 1], F32, tag="rc")
                    nc.vector.reciprocal(rc[:], sm[:])
                    dg = dgp.tile([128, 128], BF16, tag="dg")
                    nc.vector.tensor_scalar_mul(dg[:], ident[:], rc[:, 0:1])
                    # normalized transpose: PT[k,q] = P^T @ diag(1/sum)
                    pt_sb = ptp.tile([128, 1152], BF16, tag="pt")
                    for ci, chunk in enumerate(chunks):
                        w = len(chunk) * 128
                        ptps = ps.tile([128, 512], F32, tag="ps")
                        for t in range(len(chunk)):
                            nc.tensor.matmul(
                                ptps[:, t * 128 : (t + 1) * 128],
                                p_sb[:, ci * 384 + t * 128 : ci * 384 + (t + 1) * 128],
                                dg[:],
                                start=True,
                                stop=True,
                            )
                        nc.any.tensor_copy(
                            pt_sb[:, ci * 384 : ci * 384 + w], ptps[:, 0:w]
                        )
                    # PV
                    for mi, kb in enumerate(kbs):
                        ci, t = mi // 3, mi % 3
                        nc.tensor.matmul(
                            aps[:],
                            v_sb[:, kb, :],
                            pt_sb[:, ci * 384 + t * 128 : ci * 384 + (t + 1) * 128],
                            start=(mi == 0),
                            stop=(mi == nkb - 1),
                        )
                nc.any.tensor_copy(attn[:, h, qb : qb + 128], aps[:])

            # partial O-projection for this q-block (keeps dense work in the mix)
            if True:
                o_sb = op_.tile([128, 2048], F32, tag="o")
                for dn in range(4):
                    ops = ps.tile([128, 512], F32, tag="ps")
                    for f in range(HPC):
                        nc.tensor.matmul(
                            ops[:],
                            attn[:, f, qc * 128 : (qc + 1) * 128],
                            wo_t[f // 2][:, f % 2, dn * 512 : (dn + 1) * 512],
                            start=(f == 0),
                            stop=(f == HPC - 1),
                        )
                    nc.any.tensor_copy(o_sb[:, dn * 512 : (dn + 1) * 512], ops[:])
                    nc.sync.dma_start(
                        out_d[qc * 128 : (qc + 1) * 128, dn * 512 : (dn + 1) * 512],
                        o_sb[:, dn * 512 : (dn + 1) * 512],
                    )

    nc.compile()
    return nc


def _prep_core(inputs, c):
    x = inputs["x"]
    cos, sin = np.asarray(inputs["cos"]), np.asarray(inputs["sin"])
    mask = np.asarray(inputs["mask"])
    wq = np.asarray(inputs["wq"], dtype=np.float32)
    wk = np.asarray(inputs["wk"], dtype=np.float32)
    wv = np.asarray(inputs["wv"], dtype=np.float32)
    wo = np.asarray(inputs["wo"], dtype=np.float32)
    bf = ml_dtypes.bfloat16
    b, g = c // 4, c % 4

    # x[b] transposed -> [128p, cq, dc, 512]
    xt = np.asarray(x[b], dtype=np.float32).T  # [dim, S]
    xt = xt.reshape(NDC, 128, 4, 512).transpose(1, 2, 0, 3)
    xt = np.ascontiguousarray(xt).reshape(128, 4 * NDC * 512).astype(bf)

    # wq slice for heads 4g..4g+3 (SCALE folded), [p, hpair, dc, 256]
    wqs = (wq[:, g * 512 : (g + 1) * 512] * SCALE).reshape(NDC, 128, 2, 256)
    wqs = np.ascontiguousarray(wqs.transpose(1, 2, 0, 3)).reshape(128, 2 * NDC * 256)
    # wk|wv slice for kv head g: [p, dc, 256] with cols [wk 128 | wv 128]
    wkv = np.concatenate(
        [wk[:, g * 128 : (g + 1) * 128], wv[:, g * 128 : (g + 1) * 128]], axis=1
    )
    wkv = np.ascontiguousarray(wkv.reshape(NDC, 128, 256).transpose(1, 0, 2)).reshape(
        128, NDC * 256
    )
    # wo rows for this core's heads: [p, f2(2 within pair), ...] tiles [128,2,2048]
    wos = wo[g * 512 : (g + 1) * 512].reshape(2, 2, 128, 2048).transpose(2, 0, 1, 3)
    wos = np.ascontiguousarray(wos).reshape(128, 2 * 2 * 2048)

    fm = np.ascontiguousarray((cos - sin).T, dtype=np.float32)
    fp_ = np.ascontiguousarray((cos + sin).T, dtype=np.float32)
    t0 = np.ascontiguousarray(mask[WINDOW : WINDOW + 128, 0:128], dtype=np.float32)
    t8 = np.ascontiguousarray(mask[0:128, 0:128], dtype=np.float32)

    return {
        "xt": xt, "wq": wqs.astype(bf), "wkv": wkv.astype(bf), "wo": wos.astype(bf),
        "fm": fm, "fp": fp_, "t0": t0, "t8": t8,
        "ident": np.eye(128, dtype=np.float32).astype(bf),
    }


def kernel(**inputs) -> np.ndarray:
    if "nc" not in _cache:
        _cache["nc"] = _build()
    nc = _cache["nc"]
    in_maps = [_prep_core(inputs, c) for c in range(8)]
    res = run_bass_kernel_spmd(nc, in_maps, core_ids=list(range(8)))
    out = np.zeros((B, S, DIM), dtype=np.float32)
    for c in range(8):
        out[c // 4] += res.results[c]["out"]
    return out


# revision 21
# speedup vs baseline: 1.6246x; 1.0006x over previous
"""Distributed Bass kernel for sliding-window GQA attention on 8 TRN2 NeuronCores.

Problem: B=2, S=2048, DIM=2048, H=16, KVH=4, HD=128, WINDOW=1024 (causal
sliding window), nonstandard RoPE producing 1.5*HD score features.

Sharding (tensor-parallel on the kv-head axis, data-parallel on batch —
no collectives): core c owns (batch, kv-group) = (c//4, c%4): its 4 q-heads
and 1 kv head over the full 2048-row sequence. wq/wk/wv are column-sharded
by kv group, wo row-sharded. Each core emits a PARTIAL output projection
(its 4 heads x its wo rows); the host sums the 4 partials per batch while
unsharding — replacing the all-reduce.

Per core: Q/K/V projections + rope scaling, block-sparse sliding-window
attention in global coordinates (k-blocks max(0,qc-8)..qc per 128-row
q-block qc), unnormalized exp softmax (bounded scores, no max pass), a
transpose-by-matmul against diag(1/rowsum) that normalizes for free, PV,
and the partial O-projection.
"""
import numpy as np
import ml_dtypes

import concourse.tile as tile
from concourse import bacc, mybir
from concourse.bass_utils import run_bass_kernel_spmd
from contextlib import ExitStack

F32 = mybir.dt.float32
BF16 = mybir.dt.bfloat16
EXP = mybir.ActivationFunctionType.Exp

B, S, DIM = 2, 2048, 2048
H, KVH, HD = 16, 4, 128
HPC = H // KVH  # heads per core (4)
WINDOW = 1024
SCALE = HD ** -0.5
NDC = DIM // 128  # 16 dim chunks
NQC = S // 128    # 16 q blocks

_cache = {}


def _kblocks(qc):
    return list(range(max(0, qc - 8), qc + 1))


def _build():
    nc = bacc.Bacc("TRN2", target_bir_lowering=False, debug=False, num_devices=8)

    xt_d = nc.dram_tensor("xt", [128, 4 * NDC * 512], BF16, kind="ExternalInput")
    wq_d = nc.dram_tensor("wq", [128, 2 * NDC * 256], BF16, kind="ExternalInput")
    wkv_d = nc.dram_tensor("wkv", [128, NDC * 256], BF16, kind="ExternalInput")
    wo_d = nc.dram_tensor("wo", [128, 2 * 2 * 2048], BF16, kind="ExternalInput")
    fm_d = nc.dram_tensor("fm", [64, S], F32, kind="ExternalInput")
    fp_d = nc.dram_tensor("fp", [64, S], F32, kind="ExternalInput")
    t0_d = nc.dram_tensor("t0", [128, 128], F32, kind="ExternalInput")
    t8_d = nc.dram_tensor("t8", [128, 128], F32, kind="ExternalInput")
    id_d = nc.dram_tensor("ident", [128, 128], BF16, kind="ExternalInput")
    out_d = nc.dram_tensor("out", [S, DIM], F32, kind="ExternalOutput")

    with tile.TileContext(nc) as tc, ExitStack() as ctx:
        xp = ctx.enter_context(tc.tile_pool(name="xp", bufs=3))
        wp = ctx.enter_context(tc.tile_pool(name="wp", bufs=3))
        cp = ctx.enter_context(tc.tile_pool(name="cp", bufs=1))
        qp = ctx.enter_context(tc.tile_pool(name="qp", bufs=1))
        kp = ctx.enter_context(tc.tile_pool(name="kp", bufs=1))
        vp = ctx.enter_context(tc.tile_pool(name="vp", bufs=1))
        pp = ctx.enter_context(tc.tile_pool(name="pp", bufs=3))
        ptp = ctx.enter_context(tc.tile_pool(name="ptp", bufs=3))
        dgp = ctx.enter_context(tc.tile_pool(name="dgp", bufs=2))
        smp = ctx.enter_context(tc.tile_pool(name="smp", bufs=8))
        ap_ = ctx.enter_context(tc.tile_pool(name="ap", bufs=1))
        op_ = ctx.enter_context(tc.tile_pool(name="op", bufs=2))
        ps = ctx.enter_context(tc.tile_pool(name="ps", bufs=5, space="PSUM"))
        sps = ctx.enter_context(tc.tile_pool(name="sps", bufs=3, space="PSUM"))

        # ---- weights for phase 1 first (prologue-critical DMA order) ----
        wkv_t = wp.tile([128, NDC, 256], BF16, tag="w")  # cols: [wk 128 | wv 128]
        for i in range(2):
            nc.sync.dma_start(
                wkv_t[:, i * 8 : (i + 1) * 8, :],
                wkv_d[:, i * 8 * 256 : (i + 1) * 8 * 256],
            )
        wq_t = None  # allocated after the first x chunk's DMAs

        q1 = qp.tile([128, HPC, S], BF16, tag="q1")
        q2 = qp.tile([64, HPC, S], BF16, tag="q2")
        k1 = kp.tile([128, S], BF16, tag="k1")
        k2 = kp.tile([64, S], BF16, tag="k2")
        v_sb = vp.tile([128, NQC, 128], BF16, tag="v")

        # ---- fused projections + attention + O-proj per column-quarter ----
        fm = fp = t0 = t8 = ident = wo_t = None
        attn = ap_.tile([128, HPC, S], BF16, tag="attn")
        for cq in range(4):
            x_q = xp.tile([128, NDC, 512], BF16, tag="x")
            ndg = 8 if cq == 0 else 4
            w_dg = NDC // ndg
            for dg in range(ndg):
                nc.sync.dma_start(
                    x_q[:, dg * w_dg : (dg + 1) * w_dg, :],
                    xt_d[
                        :,
                        cq * NDC * 512 + dg * w_dg * 512 : cq * NDC * 512
                        + (dg + 1) * w_dg * 512,
                    ],
                )
            if cq == 0:
                # wq + constants ride after the first x chunk (not prologue-critical)
                wq_t = [
                    wp.tile([128, NDC, 256], BF16, tag="w", name=f"wq{i}")
                    for i in range(2)
                ]
                for i in range(2):
                    nc.sync.dma_start(
                        wq_t[i][:], wq_d[:, i * NDC * 256 : (i + 1) * NDC * 256]
                    )
                fm = cp.tile([64, S], F32, tag="fm")
                nc.sync.dma_start(fm[:], fm_d[:, :])
                fp = cp.tile([64, S], F32, tag="fp")
                nc.sync.dma_start(fp[:], fp_d[:, :])
                t0 = cp.tile([128, 128], F32, tag="t0")
                nc.sync.dma_start(t0[:], t0_d[:, :])
                t8 = cp.tile([128, 128], F32, tag="t8")
                nc.sync.dma_start(t8[:], t8_d[:, :])
                ident = cp.tile([128, 128], BF16, tag="ident")
                nc.sync.dma_start(ident[:], id_d[:, :])
            cs = slice(cq * 512, (cq + 1) * 512)
            fmc, fpc = fm[:, cs], fp[:, cs]

            kps = ps.tile([128, 512], F32, tag="ps")
            for dc in range(NDC):
                nc.tensor.matmul(
                    kps[:],
                    wkv_t[:, dc, 0:128],
                    x_q[:, dc, :],
                    start=(dc == 0),
                    stop=(dc == NDC - 1),
                )
            nc.vector.tensor_mul(k1[0:64, cs], kps[0:64, :], fmc)
            nc.vector.tensor_mul(k1[64:128, cs], kps[0:64, :], fpc)
            nc.scalar.copy(k2[:, cs], kps[64:128, :])

            vps = ps.tile([128, 512], F32, tag="ps")
            for kb4 in range(4):
                kb = cq * 4 + kb4
                for dc in range(NDC):
                    nc.tensor.matmul(
                        vps[:, kb4 * 128 : (kb4 + 1) * 128],
                        x_q[:, dc, kb4 * 128 : (kb4 + 1) * 128],
                        wkv_t[:, dc, 128:256],
                        start=(dc == 0),
                        stop=(dc == NDC - 1),
                    )
            for kb4 in range(4):
                nc.any.tensor_copy(
                    v_sb[:, cq * 4 + kb4, :], vps[:, kb4 * 128 : (kb4 + 1) * 128]
                )

            for h in range(HPC):
                qps = ps.tile([128, 512], F32, tag="ps")
                for dc in range(NDC):
                    nc.tensor.matmul(
                        qps[:],
                        wq_t[h // 2][:, dc, (h % 2) * 128 : (h % 2 + 1) * 128],
                        x_q[:, dc, :],
                        start=(dc == 0),
                        stop=(dc == NDC - 1),
                    )
                nc.vector.tensor_mul(q1[0:64, h, cs], qps[0:64, :], fmc)
                nc.vector.tensor_mul(q1[64:128, h, cs], qps[0:64, :], fpc)
                nc.scalar.copy(q2[:, h, cs], qps[64:128, :])

        # ---- attention + per-group O-projection ----
        wo_t = [
            wp.tile([128, 2, 2048], BF16, tag="w", name=f"wo{i}") for i in range(2)
        ]
        for i in range(2):
            nc.sync.dma_start(wo_t[i][:], wo_d[:, i * 4096 : (i + 1) * 4096])
        for qc in reversed(range(NQC)):
            qb = qc * 128
            for h in range(HPC):
                if True:
                    aps = ps.tile([128, 128], F32, tag="ps")
                    kbs = _kblocks(qc)
                    nkb = len(kbs)
                    chunks = [kbs[i : i + 3] for i in range(0, nkb, 3)]
                    schunks = []
                    for chunk in chunks:
                        w = len(chunk) * 128
                        sp = sps.tile([128, 384], F32, tag="s")
                        lo = chunk[0] * 128
                        nc.tensor.matmul(
                            sp[:, 0:w],
                            q1[:, h, qb : qb + 128],
                            k1[:, lo : lo + w],
                            start=True,
                            stop=False,
                        )
                        nc.tensor.matmul(
                            sp[:, 0:w],
                            q2[:, h, qb : qb + 128],
                            k2[:, lo : lo + w],
                            start=False,
                            stop=True,
                        )
                        schunks.append(sp)
                    # masks: window-tail triangle on k-block qc-8, causal on qc
                    if kbs[0] == qc - 8:
                        nc.vector.tensor_add(
                            schunks[0][:, 0:128], schunks[0][:, 0:128], t0[:]
                        )
                    dpos = (nkb - 1) % 3
                    nc.vector.tensor_add(
                        schunks[-1][:, dpos * 128 : (dpos + 1) * 128],
                        schunks[-1][:, dpos * 128 : (dpos + 1) * 128],
                        t8[:],
                    )
                    # exp + row sums
                    p_sb = pp.tile([128, 1152], BF16, tag="p")
                    acc = smp.tile([128, 3], F32, tag="acc")
                    for ci, chunk in enumerate(chunks):
                        w = len(chunk) * 128
                        nc.scalar.activation(
                            p_sb[:, ci * 384 : ci * 384 + w],
                            schunks[ci][:, 0:w],
                            EXP,
                            accum_out=acc[:, ci : ci + 1],
                        )
                    sm = smp.tile([128, 1], F32, tag="sm")
                    if len(chunks) == 1:
                        nc.vector.tensor_copy(sm[:], acc[:, 0:1])
                    else:
                        nc.vector.tensor_add(sm[:], acc[:, 0:1], acc[:, 1:2])
                        if len(chunks) == 3:
                            nc.vector.tensor_add(sm[:], sm[:], acc[:, 2:3])
                    rc = smp.tile([128, 1], F32, tag="rc")
                    nc.vector.reciprocal(rc[:], sm[:])
                    dg = dgp.tile([128, 128], BF16, tag="dg")
                    nc.vector.tensor_scalar_mul(dg[:], ident[:], rc[:, 0:1])
                    # normalized transpose: PT[k,q] = P^T @ diag(1/sum)
                    pt_sb = ptp.tile([128, 1152], BF16, tag="pt")
                    for ci, chunk in enumerate(chunks):
                        w = len(chunk) * 128
                        ptps = ps.tile([128, 512], F32, tag="ps")
                        for t in range(len(chunk)):
                            nc.tensor.matmul(
                                ptps[:, t * 128 : (t + 1) * 128],
                                p_sb[:, ci * 384 + t * 128 : ci * 384 + (t + 1) * 128],
                                dg[:],
                                start=True,
                                stop=True,
                            )
                        nc.any.tensor_copy(
                            pt_sb[:, ci * 384 : ci * 384 + w], ptps[:, 0:w]
                        )
                    # PV
                    for mi, kb in enumerate(kbs):
                        ci, t = mi // 3, mi % 3
                        nc.tensor.matmul(
                            aps[:],
                            v_sb[:, kb, :],
                            pt_sb[:, ci * 384 + t * 128 : ci * 384 + (t + 1) * 128],
                            start=(mi == 0),
                            stop=(mi == nkb - 1),
                        )
                nc.any.tensor_copy(attn[:, h, qb : qb + 128], aps[:])

            # partial O-projection for this q-block (keeps dense work in the mix)
            if True:
                o_sb = op_.tile([128, 2048], F32, tag="o")
                for dn in range(4):
                    ops = ps.tile([128, 512], F32, tag="ps")
                    for f in range(HPC):
                        nc.tensor.matmul(
                            ops[:],
                            attn[:, f, qc * 128 : (qc + 1) * 128],
                            wo_t[f // 2][:, f % 2, dn * 512 : (dn + 1) * 512],
                            start=(f == 0),
                            stop=(f == HPC - 1),
                        )
                    nc.any.tensor_copy(o_sb[:, dn * 512 : (dn + 1) * 512], ops[:])
                    nc.sync.dma_start(
                        out_d[qc * 128 : (qc + 1) * 128, dn * 512 : (dn + 1) * 512],
                        o_sb[:, dn * 512 : (dn + 1) * 512],
                    )

    nc.compile()
    return nc


def _prep_core(inputs, c):
    x = inputs["x"]
    cos, sin = np.asarray(inputs["cos"]), np.asarray(inputs["sin"])
    mask = np.asarray(inputs["mask"])
    wq = np.asarray(inputs["wq"], dtype=np.float32)
    wk = np.asarray(inputs["wk"], dtype=np.float32)
    wv = np.asarray(inputs["wv"], dtype=np.float32)
    wo = np.asarray(inputs["wo"], dtype=np.float32)
    bf = ml_dtypes.bfloat16
    b, g = c // 4, c % 4

    # x[b] transposed -> [128p, cq, dc, 512]
    xt = np.asarray(x[b], dtype=np.float32).T  # [dim, S]
    xt = xt.reshape(NDC, 128, 4, 512).transpose(1, 2, 0, 3)
    xt = np.ascontiguousarray(xt).reshape(128, 4 * NDC * 512).astype(bf)

    # wq slice for heads 4g..4g+3 (SCALE folded), [p, hpair, dc, 256]
    wqs = (wq[:, g * 512 : (g + 1) * 512] * SCALE).reshape(NDC, 128, 2, 256)
    wqs = np.ascontiguousarray(wqs.transpose(1, 2, 0, 3)).reshape(128, 2 * NDC * 256)
    # wk|wv slice for kv head g: [p, dc, 256] with cols [wk 128 | wv 128]
    wkv = np.concatenate(
        [wk[:, g * 128 : (g + 1) * 128], wv[:, g * 128 : (g + 1) * 128]], axis=1
    )
    wkv = np.ascontiguousarray(wkv.reshape(NDC, 128, 256).transpose(1, 0, 2)).reshape(
        128, NDC * 256
    )
    # wo rows for this core's heads: [p, f2(2 within pair), ...] tiles [128,2,2048]
    wos = wo[g * 512 : (g + 1) * 512].reshape(2, 2, 128, 2048).transpose(2, 0, 1, 3)
    wos = np.ascontiguousarray(wos).reshape(128, 2 * 2 * 2048)

    fm = np.ascontiguousarray((cos - sin).T, dtype=np.float32)
    fp_ = np.ascontiguousarray((cos + sin).T, dtype=np.float32)
    t0 = np.ascontiguousarray(mask[WINDOW : WINDOW + 128, 0:128], dtype=np.float32)
    t8 = np.ascontiguousarray(mask[0:128, 0:128], dtype=np.float32)

    return {
        "xt": xt, "wq": wqs.astype(bf), "wkv": wkv.astype(bf), "wo": wos.astype(bf),
        "fm": fm, "fp": fp_, "t0": t0, "t8": t8,
        "ident": np.eye(128, dtype=np.float32).astype(bf),
    }


def kernel(**inputs) -> np.ndarray:
    if "nc" not in _cache:
        _cache["nc"] = _build()
    nc = _cache["nc"]
    in_maps = [_prep_core(inputs, c) for c in range(8)]
    res = run_bass_kernel_spmd(nc, in_maps, core_ids=list(range(8)))
    out = np.zeros((B, S, DIM), dtype=np.float32)
    for c in range(8):
        out[c // 4] += res.results[c]["out"]
    return out
